# revision 10
# baseline (speedup 1.0000x reference)
"""CRF (dense projection + Viterbi decode) on 8 Trainium2 NeuronCores.

Strategy: data-parallel over batch (8 batches per core).
Per core:
  Phase 1: potentials = x @ W + bias (+boundary cols) on the PE, output in
           [u-partition, t-free] orientation (x fed pre-transposed from host).
  Phase 2: Viterbi forward scan, all-DVE, using tensor_reduce with
           apply_transpose (32x32 reshape-block) to reduce over the
           transition-source tag axis that lives on partitions.
           Layout: partition = (j=batch%4, vc=tag&31), free = (q=batch//4,
           vr=tag>>5, ...).
  Phase 3: backpointers recomputed in bulk (t-chunks of 8): recompute scores
           bitwise-identically, 32x32-block stream transpose, compare against
           stored per-step maxima, encode argmin-index via a monotone iota
           code, grouped max-reduce, decode to uint16. Chunk start (transpose,
           compare, Pool code-mult) and finish (reduce, decode) are emitted 8
           steps apart so the Pool mult never stalls the DVE pipeline.
  Phase 4: bulk 32x32 stream-transpose of the backpointer tile, then one
           SBUF->SBUF scatter DMA per (batch, tag-half) into per-batch
           partition rows; a sequential gpsimd indirect_copy chain walks the
           backpointers (one core-group of 16 partitions per batch).

All DMAs are issued from the SP/Activation sequencers (hardware DGE) so no
compute engine burns time on software descriptor generation.

Host side only shards/reshapes/transposes inputs and concatenates outputs.
"""

import numpy as np
from contextlib import ExitStack

B, T, F, U = 64, 512, 1024, 64
NB = 8  # batches per core
NCORE = 8

_CACHE = {}


def _build_program():
    import concourse.bass as bass
    import concourse.bacc as bacc
    import concourse.mybir as mybir
    import concourse.tile as tile

    dt = mybir.dt
    AL = mybir.AluOpType
    AX = mybir.AxisListType
    ACT = mybir.ActivationFunctionType

    nc = bacc.Bacc("TRN2", target_bir_lowering=False, debug=False,
                   num_devices=NCORE)

    # ---- DRAM I/O ----
    d_xt = nc.dram_tensor("xt", [NB, F, T], dt.float32, kind="ExternalInput")
    d_wkp = nc.dram_tensor("wkp", [128, 512], dt.float32, kind="ExternalInput")
    d_bias = nc.dram_tensor("biasc", [64, 1], dt.float32, kind="ExternalInput")
    d_lb = nc.dram_tensor("lbc", [64, 1], dt.float32, kind="ExternalInput")
    d_rb = nc.dram_tensor("rbc", [64, 1], dt.float32, kind="ExternalInput")
    d_crep = nc.dram_tensor("crep", [128, 128], dt.float32, kind="ExternalInput")
    d_iot = nc.dram_tensor("iot", [128, 64], dt.float32, kind="ExternalInput")
    d_tags = nc.dram_tensor("tags", [NB, T], dt.int32, kind="ExternalOutput")
    DEBUG = bool(int(__import__("os").environ.get("CRF_DEBUG", "0")))
    if DEBUG:
        d_gam = nc.dram_tensor("dbg_gam", [128, 2048], dt.float32,
                               kind="ExternalOutput")
        d_beta = nc.dram_tensor("dbg_beta", [128, 2052], dt.float32,
                                kind="ExternalOutput")
        d_pot = nc.dram_tensor("dbg_pot", [128, 2048], dt.float32,
                               kind="ExternalOutput")

    TB = 513          # beta t-slots (1..512 used)

    with tile.TileContext(nc) as tc, ExitStack() as ctx:
        cpool = ctx.enter_context(tc.tile_pool(name="consts", bufs=1))
        st = ctx.enter_context(tc.tile_pool(name="state", bufs=1))
        xpool = ctx.enter_context(tc.tile_pool(name="xs", bufs=8))
        ppool = ctx.enter_context(tc.tile_pool(name="ps", bufs=8, space="PSUM"))
        spool = ctx.enter_context(tc.tile_pool(name="scan", bufs=2))
        vpool = ctx.enter_context(tc.tile_pool(name="conv", bufs=2))

        wk = cpool.tile([128, 512], dt.float32, tag="wk")
        crep = cpool.tile([128, 128], dt.float32, tag="crep")
        iot = cpool.tile([128, 64], dt.float32, tag="iot")
        biasc = cpool.tile([64, 1], dt.float32, tag="biasc")
        lbc = cpool.tile([64, 1], dt.float32, tag="lbc")
        rbc = cpool.tile([64, 1], dt.float32, tag="rbc")
        zcol = cpool.tile([128, 1], dt.float32, tag="zcol")
        zidx = cpool.tile([128, 1], dt.uint16, tag="zidx")

        pot = st.tile([128, 2048], dt.float32, tag="pot")    # (q,uh,t<512)
        gam = st.tile([128, 2048], dt.float32, tag="gam")    # (q,uh,t<512)
        beta = st.tile([128, 2 * 2 * TB], dt.float32, tag="beta")  # (q,uh,t<=512)
        # bpu[(j,uc), 1024q + 64*((t-1)>>5) + 32uh + ((t-1)&31)] =
        #   bp[b=(q,j), t, u=32uh+uc], t in 1..512 (all 2048 slots used)
        bpu = st.tile([128, 2048], dt.uint16, tag="bpu")
        ssring = st.tile([128, 16 * 256], dt.float32, tag="ssring")
        stt = st.tile([128, 2048], dt.float32, tag="stt")
        pkks = [st.tile([128, 2048], dt.bfloat16, tag=f"pkk{i}", name=f"pkk{i}")
                for i in range(2)]
        pk2s = [st.tile([128, 2048], dt.bfloat16, tag=f"pk2{i}", name=f"pk2{i}")
                for i in range(2)]
        r4 = st.tile([128, 64], dt.float32, tag="r4")
        bpks = [st.tile([128, 32], dt.float32, tag=f"bpk{i}", name=f"bpk{i}")
                for i in range(2)]
        btp_all = st.tile([128, 2048], dt.uint16, tag="btp_all")
        # brow[16b, 1024*((t-1)&31) + 64*((t-1)>>5) + u] = bp[b, t, u]
        brow = st.tile([128, 32 * 1024], dt.uint16, tag="brow")
        tags16 = st.tile([128, 4 * 512], dt.uint16, tag="tags16")
        tagsi = st.tile([128, 512], dt.int32, tag="tagsi")

        def A(tl, p0, np_, f0, dims):
            full = tl[:]
            pitch = full.ap[0][0]
            return bass.AP(full.tensor, full.offset + p0 * pitch + f0,
                           [[pitch, np_]] + [list(d) for d in dims])

        # ---- constant loads / inits (HWDGE via SP) ----
        nc.sync.dma_start(wk[:], d_wkp[:])
        nc.sync.dma_start(crep[:], d_crep[:])
        nc.sync.dma_start(iot[:], d_iot[:])
        nc.sync.dma_start(biasc[:], d_bias[:])
        nc.sync.dma_start(lbc[:], d_lb[:])
        nc.sync.dma_start(rbc[:], d_rb[:])
        nc.vector.memset(zcol[:], 0.0)
        nc.vector.memset(zidx[:], 0)
        nc.vector.memset(tags16[:], 0)
        nc.gpsimd.memset(brow[:], 0)
        if DEBUG:
            nc.vector.memset(beta[:], 0.0)

        # ---- Phase 1: potentials ----
        # PE warmup; also funnels the wk-DMA dependency into PE program order
        # so later matmuls need only their x-tile DMA wait (PE matmul HW
        # decode supports a single sync-wait).
        for b in range(NB):
            q, j = b >> 2, b & 3
            pp = ppool.tile([64, 512], dt.float32, tag="pp")
            if b == 0:
                nc.tensor.matmul(pp[0:64, 0:1], wk[:, 0:64], wk[:, 0:1],
                                 start=True, stop=True)
            for kc in range(8):
                xt = xpool.tile([128, 512], dt.float32, tag="xt")
                nc.sync.dma_start(xt[:], d_xt[b, kc * 128:(kc + 1) * 128, :])
                nc.tensor.matmul(pp[:], wk[:, kc * 64:(kc + 1) * 64], xt[:],
                                 start=(kc == 0), stop=(kc == 7))
            potb = vpool.tile([64, 512], dt.float32, tag="potb")
            nc.vector.tensor_scalar_add(potb[:], pp[:], biasc[:])
            nc.vector.tensor_add(potb[:, 0:1], potb[:, 0:1], lbc[:])
            nc.vector.tensor_add(potb[:, 511:512], potb[:, 511:512], rbc[:])
            for uh in range(2):
                dst = A(pot, 32 * j, 32, 1024 * q + 512 * uh, [[1, 512]])
                nc.scalar.dma_start(dst, potb[32 * uh:32 * uh + 32, :])

        # gamma_0 = pot_0
        nc.vector.tensor_copy(
            out=A(gam, 0, 128, 0, [[1024, 2], [512, 2]]),
            in_=A(pot, 0, 128, 0, [[1024, 2], [512, 2]]))

        # ---- Phase 2: forward scan with split-pipelined bp chunks ----
        def _bp_start(ci, t0, L):
            pkk = pkks[ci % 2]
            n = L * 256
            rbase = ((t0 - 1) % 16) * 256
            # flat 32-block transpose of the ring chunk: stt layout becomes
            # (t, q, vr, uh, c) with c = former partition index
            nc.vector.transpose(out=stt[:, :n],
                                in_=ssring[:, rbase:rbase + n])
            for qq in range(2):
                for vr in range(2):
                    off = qq * 128 + vr * 64
                    s_in = A(stt, 0, 128, off, [[256, L], [32, 2], [1, 32]])
                    b_in = A(beta, 0, 128, 2 * TB * qq + t0,
                             [[1, L], [TB, 2], [0, 32]])
                    p_out = A(pkk, 0, 128, off, [[256, L], [32, 2], [1, 32]])
                    nc.vector.tensor_tensor(out=p_out, in0=s_in, in1=b_in,
                                            op=AL.is_ge)
                    i_in = A(iot, 0, 128, 32 * vr,
                             [[0, L], [0, 2], [1, 32]])
                    s_out = A(pk2s[ci % 2], 0, 128, off,
                              [[256, L], [32, 2], [1, 32]])
                    nc.gpsimd.tensor_tensor(out=s_out, in0=p_out, in1=i_in,
                                            op=AL.mult)

        def _bp_finish(ci, t0, L):
            pk2 = pk2s[ci % 2]
            bpk = bpks[ci % 2]
            nc.vector.tensor_reduce(A(r4, 0, 128, 0, [[1, 8 * L]]),
                                    A(pk2, 0, 128, 0, [[32, 8 * L], [1, 32]]),
                                    AX.X, AL.max)
            # r4 flat layout: (t, q, vr, uh)
            nc.vector.tensor_tensor(
                out=A(bpk, 0, 128, 0, [[1, 4 * L]]),
                in0=A(r4, 0, 128, 0, [[8, L], [4, 2], [1, 2]]),
                in1=A(r4, 0, 128, 2, [[8, L], [4, 2], [1, 2]]),
                op=AL.max)
            # decode: v = 127 - 64*packed; bpk flat (t, q, uh)
            tc0, rr0 = (t0 - 1) >> 5, (t0 - 1) & 31
            nc.scalar.activation(
                A(bpu, 0, 128, 64 * tc0 + rr0, [[1, L], [1024, 2], [32, 2]]),
                A(bpk, 0, 128, 0, [[1, 4 * L]]),
                ACT.Copy, bias=127.0, scale=-64.0)

        pending = None  # (ci, t0, L) started but not finished
        for t in range(1, 513):
            slot = (t - 1) % 16
            sbase = slot * 256
            if t <= 511:
                in0 = A(crep, 0, 128, 0, [[0, 2], [64, 2], [1, 64]])
            else:
                in0 = A(zcol, 0, 128, 0, [[0, 2], [0, 2], [0, 64]])
            in1 = A(gam, 0, 128, t - 1, [[1024, 2], [512, 2], [0, 64]])
            nc.vector.tensor_tensor(out=ssring[:, sbase:sbase + 256],
                                    in0=in0, in1=in1, op=AL.add)
            rr = spool.tile([128, 8], dt.float32, tag="rr")
            nc.vector.tensor_reduce(rr[:],
                                    A(ssring, 0, 128, sbase, [[32, 8], [1, 32]]),
                                    AX.X, AL.max, apply_transpose=True)
            bsl = A(beta, 0, 128, t, [[2 * TB, 2], [TB, 2]])
            nc.vector.tensor_tensor(out=bsl,
                                    in0=A(rr, 0, 128, 0, [[4, 2], [1, 2]]),
                                    in1=A(rr, 0, 128, 2, [[4, 2], [1, 2]]),
                                    op=AL.max)
            if t <= 511:
                nc.vector.tensor_tensor(
                    out=A(gam, 0, 128, t, [[1024, 2], [512, 2]]),
                    in0=bsl,
                    in1=A(pot, 0, 128, t, [[1024, 2], [512, 2]]),
                    op=AL.add)

            if t % 8 == 0 and t <= 504:
                ci = t // 8 - 1
                if pending is not None:
                    _bp_finish(*pending)
                _bp_start(ci, t - 7, 8)
                pending = (ci, t - 7, 8)
            elif t == 511:
                if pending is not None:
                    _bp_finish(*pending)
                _bp_start(63, 505, 7)
                pending = (63, 505, 7)
            elif t == 512:
                _bp_finish(*pending)
                _bp_start(64, 512, 1)
                _bp_finish(64, 512, 1)
                pending = None

        # ---- Phase 4a: bulk transpose of bpu + SBUF->SBUF scatter ----
        # btp_all[(j,rr), 1024q + 64tc + 32uh + uc] = bp[b=(q,j),
        #   t=32tc+rr+1, u=32uh+uc]: per (j, rr, q) a contiguous 1024-run.
        nc.vector.transpose(out=btp_all[:], in_=bpu[:])
        brp = brow[:].ap[0][0]
        btpp = btp_all[:].ap[0][0]
        for b in range(NB):
            q, j = b >> 2, b & 3
            src = bass.AP(btp_all[:].tensor,
                          btp_all[:].offset + 32 * j * btpp + 1024 * q,
                          [[btpp, 32], [1, 1024]])
            dst = bass.AP(brow[:].tensor,
                          brow[:].offset + 16 * b * brp,
                          [[brp, 1], [1024, 32], [1, 1024]])
            nc.sync.dma_start(dst, src)

        def block_slice(t):
            off = (brow[:].offset + ((t - 1) & 31) * 1024
                   + ((t - 1) >> 5) * 64)
            return bass.AP(brow[:].tensor, off, [[brp, 128], [1, 64]])

        # ---- Phase 4b: backtrace chain (gpsimd) ----
        # chain: tags16 is 4-wide per t (indirect_copy needs out free >= 4;
        # only column 0 of each group is meaningful)
        nc.gpsimd.indirect_copy(tags16[:, 4 * 511:4 * 512], block_slice(512),
                                zidx[:], True)
        for t in range(511, 0, -1):
            nc.gpsimd.indirect_copy(tags16[:, 4 * (t - 1):4 * t],
                                    block_slice(t),
                                    tags16[:, 4 * t:4 * t + 1], True)

        if DEBUG:
            nc.sync.dma_start(d_gam[:], gam[:])
            nc.sync.dma_start(d_beta[:], beta[:])
            nc.sync.dma_start(d_pot[:], pot[:])

        t16 = tags16[:]
        nc.vector.tensor_copy(
            out=tagsi[:],
            in_=bass.AP(t16.tensor, t16.offset, [[t16.ap[0][0], 128], [4, 512]]))
        ti = tagsi[:]
        src = bass.AP(ti.tensor, ti.offset, [[16 * ti.ap[0][0], 8], [1, 512]])
        nc.sync.dma_start(d_tags[:], src)

    nc.finalize()
    return nc


def _host_inputs(x, kernel, bias, chain_kernel, left_boundary, right_boundary):
    """Build per-core input maps (host does only sharding/layout)."""
    x = np.asarray(x, dtype=np.float32)
    wk = np.asarray(kernel, dtype=np.float32)
    bias = np.asarray(bias, dtype=np.float32)
    C = np.asarray(chain_kernel, dtype=np.float32)
    lb = np.asarray(left_boundary, dtype=np.float32)
    rb = np.asarray(right_boundary, dtype=np.float32)

    wkp = wk.reshape(8, 128, 64).transpose(1, 0, 2).reshape(128, 512).copy()
    crep = np.tile(C.reshape(2, 32, 64).transpose(1, 0, 2).reshape(32, 128),
                   (4, 1)).copy()
    iota_row = 1.0 + (63.0 - np.arange(64, dtype=np.float32)) / 64.0
    iot = np.tile(iota_row[None, :], (128, 1)).copy()
    biasc = bias.reshape(64, 1).copy()
    lbc = lb.reshape(64, 1).copy()
    rbc = rb.reshape(64, 1).copy()

    in_maps = []
    for c in range(NCORE):
        xc = x[c * NB:(c + 1) * NB]                       # [8, 512, 1024]
        xt = np.ascontiguousarray(xc.transpose(0, 2, 1))  # [8, 1024, 512]
        in_maps.append({
            "xt": xt, "wkp": wkp, "biasc": biasc, "lbc": lbc, "rbc": rbc,
            "crep": crep, "iot": iot,
        })
    return in_maps


def kernel(x, kernel, bias, chain_kernel, left_boundary, right_boundary):
    from concourse.bass_utils import run_bass_kernel_spmd

    if "nc" not in _CACHE:
        _CACHE["nc"] = _build_program()
    nc = _CACHE["nc"]

    in_maps = _host_inputs(x, kernel, bias, chain_kernel,
                           left_boundary, right_boundary)
    res = run_bass_kernel_spmd(nc, in_maps, core_ids=list(range(NCORE)))
    outs = [np.asarray(r["tags"]).astype(np.int32) for r in res.results]
    return np.concatenate(outs, axis=0)


# revision 20
# speedup vs baseline: 1.0038x; 1.0038x over previous
"""CRF (dense projection + Viterbi decode) on 8 Trainium2 NeuronCores.

Strategy: data-parallel over batch (8 batches per core).
Per core:
  Phase 1: potentials = x @ W + bias (+boundary cols) on the PE, output in
           [u-partition, t-free] orientation (x fed pre-transposed from host).
  Phase 2: Viterbi forward scan, all-DVE, using tensor_reduce with
           apply_transpose (32x32 reshape-block) to reduce over the
           transition-source tag axis that lives on partitions.
           Layout: partition = (j=batch%4, vc=tag&31), free = (q=batch//4,
           vr=tag>>5, ...).
  Phase 3: backpointers recomputed in bulk (t-chunks of 8): recompute scores
           bitwise-identically, 32x32-block stream transpose, compare against
           stored per-step maxima, encode argmin-index via a monotone iota
           code, grouped max-reduce, decode to uint16. Chunk start (transpose,
           compare, Pool code-mult) and finish (reduce, decode) are emitted 8
           steps apart so the Pool mult never stalls the DVE pipeline.
  Phase 4: bulk 32x32 stream-transpose of the backpointer tile, then one
           SBUF->SBUF scatter DMA per (batch, tag-half) into per-batch
           partition rows; a sequential gpsimd indirect_copy chain walks the
           backpointers (one core-group of 16 partitions per batch).

All DMAs are issued from the SP/Activation sequencers (hardware DGE) so no
compute engine burns time on software descriptor generation.

Host side only shards/reshapes/transposes inputs and concatenates outputs.
"""

import numpy as np
from contextlib import ExitStack

B, T, F, U = 64, 512, 1024, 64
NB = 8  # batches per core
NCORE = 8

_CACHE = {}


def _build_program():
    import concourse.bass as bass
    import concourse.bacc as bacc
    import concourse.mybir as mybir
    import concourse.tile as tile

    dt = mybir.dt
    AL = mybir.AluOpType
    AX = mybir.AxisListType
    ACT = mybir.ActivationFunctionType

    nc = bacc.Bacc("TRN2", target_bir_lowering=False, debug=False,
                   num_devices=NCORE)

    # ---- DRAM I/O ----
    d_xt = nc.dram_tensor("xt", [NB, F, T], dt.float32, kind="ExternalInput")
    d_wkp = nc.dram_tensor("wkp", [128, 512], dt.float32, kind="ExternalInput")
    d_bias = nc.dram_tensor("biasc", [64, 1], dt.float32, kind="ExternalInput")
    d_lb = nc.dram_tensor("lbc", [64, 1], dt.float32, kind="ExternalInput")
    d_rb = nc.dram_tensor("rbc", [64, 1], dt.float32, kind="ExternalInput")
    d_crep = nc.dram_tensor("crep", [128, 128], dt.float32, kind="ExternalInput")
    d_iot = nc.dram_tensor("iot", [128, 64], dt.float32, kind="ExternalInput")
    d_tags = nc.dram_tensor("tags", [NB, T], dt.int32, kind="ExternalOutput")
    DEBUG = bool(int(__import__("os").environ.get("CRF_DEBUG", "0")))
    if DEBUG:
        d_gam = nc.dram_tensor("dbg_gam", [128, 4096], dt.float32,
                               kind="ExternalOutput")
        d_beta = nc.dram_tensor("dbg_beta", [128, 2052], dt.float32,
                                kind="ExternalOutput")
        d_pot = nc.dram_tensor("dbg_pot", [128, 2048], dt.float32,
                               kind="ExternalOutput")

    TB = 513          # beta t-slots (1..512 used)

    with tile.TileContext(nc) as tc, ExitStack() as ctx:
        cpool = ctx.enter_context(tc.tile_pool(name="consts", bufs=1))
        st = ctx.enter_context(tc.tile_pool(name="state", bufs=1))
        xpool = ctx.enter_context(tc.tile_pool(name="xs", bufs=8))
        ppool = ctx.enter_context(tc.tile_pool(name="ps", bufs=8, space="PSUM"))
        spool = ctx.enter_context(tc.tile_pool(name="scan", bufs=2))
        vpool = ctx.enter_context(tc.tile_pool(name="conv", bufs=2))

        wk = cpool.tile([128, 512], dt.float32, tag="wk")
        crep = cpool.tile([128, 128], dt.float32, tag="crep")
        iot = cpool.tile([128, 64], dt.float32, tag="iot")
        biasc = cpool.tile([64, 1], dt.float32, tag="biasc")
        lbc = cpool.tile([64, 1], dt.float32, tag="lbc")
        rbc = cpool.tile([64, 1], dt.float32, tag="rbc")
        zcol = cpool.tile([128, 1], dt.float32, tag="zcol")
        zidx = cpool.tile([128, 1], dt.uint16, tag="zidx")

        pot = st.tile([128, 2048], dt.float32, tag="pot")    # (q,uh,t<512)
        # gam2[(j,x), 2048q + 1024uh + 512vr + t] = gamma_t[b=(q,j),
        #   tag=(vr,x)] replicated over uh so the scan add reads (uh,vr) as
        #   one stride-512 dim.
        gam2 = st.tile([128, 4096], dt.float32, tag="gam2")
        beta = st.tile([128, 2 * 2 * TB], dt.float32, tag="beta")  # (q,uh,t<=512)
        # bpu[(j,uc), 1024q + 64*((t-1)>>5) + 32uh + ((t-1)&31)] =
        #   bp[b=(q,j), t, u=32uh+uc], t in 1..512 (all 2048 slots used)
        bpu = st.tile([128, 2048], dt.uint16, tag="bpu")
        # ssring slot cols: 128q + 64uh + 32vr + uc (per-step scores)
        ssring = st.tile([128, 16 * 256], dt.float32, tag="ssring")
        stts = [st.tile([128, 2048], dt.float32, tag=f"stt{i}", name=f"stt{i}")
                for i in range(2)]
        pkks = [st.tile([128, 2048], dt.bfloat16, tag=f"pkk{i}", name=f"pkk{i}")
                for i in range(2)]
        pk2s = [st.tile([128, 2048], dt.bfloat16, tag=f"pk2{i}", name=f"pk2{i}")
                for i in range(2)]
        r4 = st.tile([128, 64], dt.float32, tag="r4")
        btp_all = st.tile([128, 2048], dt.uint16, tag="btp_all")
        # brow[16b, 1024*((t-1)&31) + 64*((t-1)>>5) + u] = bp[b, t, u]
        brow = st.tile([128, 32 * 1024], dt.uint16, tag="brow")
        tags16 = st.tile([128, 4 * 512], dt.uint16, tag="tags16")
        tagsi = st.tile([128, 512], dt.int32, tag="tagsi")

        def A(tl, p0, np_, f0, dims):
            full = tl[:]
            pitch = full.ap[0][0]
            return bass.AP(full.tensor, full.offset + p0 * pitch + f0,
                           [[pitch, np_]] + [list(d) for d in dims])

        # ---- constant loads / inits (HWDGE via SP) ----
        nc.sync.dma_start(wk[:], d_wkp[:])
        nc.sync.dma_start(crep[:], d_crep[:])
        nc.sync.dma_start(iot[:], d_iot[:])
        nc.sync.dma_start(biasc[:], d_bias[:])
        nc.sync.dma_start(lbc[:], d_lb[:])
        nc.sync.dma_start(rbc[:], d_rb[:])
        nc.vector.memset(zcol[:], 0.0)
        nc.vector.memset(zidx[:], 0)
        nc.vector.memset(tags16[:], 0)
        nc.gpsimd.memset(brow[:], 0)
        if DEBUG:
            nc.vector.memset(beta[:], 0.0)

        # ---- Phase 1: potentials ----
        # PE warmup; also funnels the wk-DMA dependency into PE program order
        # so later matmuls need only their x-tile DMA wait (PE matmul HW
        # decode supports a single sync-wait).
        for b in range(NB):
            q, j = b >> 2, b & 3
            pp = ppool.tile([64, 512], dt.float32, tag="pp")
            if b == 0:
                nc.tensor.matmul(pp[0:64, 0:1], wk[:, 0:64], wk[:, 0:1],
                                 start=True, stop=True)
            for kc in range(8):
                xt = xpool.tile([128, 512], dt.float32, tag="xt")
                nc.sync.dma_start(xt[:], d_xt[b, kc * 128:(kc + 1) * 128, :])
                nc.tensor.matmul(pp[:], wk[:, kc * 64:(kc + 1) * 64], xt[:],
                                 start=(kc == 0), stop=(kc == 7))
            potb = vpool.tile([64, 512], dt.float32, tag="potb")
            nc.vector.tensor_scalar_add(potb[:], pp[:], biasc[:])
            nc.vector.tensor_add(potb[:, 0:1], potb[:, 0:1], lbc[:])
            nc.vector.tensor_add(potb[:, 511:512], potb[:, 511:512], rbc[:])
            for uh in range(2):
                dst = A(pot, 32 * j, 32, 1024 * q + 512 * uh, [[1, 512]])
                nc.scalar.dma_start(dst, potb[32 * uh:32 * uh + 32, :])

        # gamma_0 = pot_0 (replicated over uh)
        nc.vector.tensor_copy(
            out=A(gam2, 0, 128, 0, [[2048, 2], [512, 2], [1024, 2]]),
            in_=A(pot, 0, 128, 0, [[1024, 2], [512, 2], [0, 2]]))

        # ---- Phase 2: forward scan with split-pipelined bp chunks ----
        def _bp_start(ci, t0, L):
            pkk = pkks[ci % 2]
            stt = stts[ci % 2]
            n = L * 256
            rbase = ((t0 - 1) % 16) * 256
            # flat 32-block transpose of the ring chunk: stt layout becomes
            # (t, q, uh, i) with i = 32vr + former partition index
            nc.vector.transpose(out=stt[:, :n],
                                in_=ssring[:, rbase:rbase + n])
            for qq in range(2):
                off = qq * 128
                s_in = A(stt, 0, 128, off, [[256, L], [64, 2], [1, 64]])
                b_in = A(beta, 0, 128, 2 * TB * qq + t0,
                         [[1, L], [TB, 2], [0, 64]])
                p_out = A(pkk, 0, 128, off, [[256, L], [64, 2], [1, 64]])
                nc.vector.tensor_tensor(out=p_out, in0=s_in, in1=b_in,
                                        op=AL.is_ge)
                i_in = A(iot, 0, 128, 0, [[0, L], [0, 2], [1, 64]])
                s_out = A(pk2s[ci % 2], 0, 128, off,
                          [[256, L], [64, 2], [1, 64]])
                nc.gpsimd.tensor_tensor(out=s_out, in0=p_out, in1=i_in,
                                        op=AL.mult)

        def _bp_finish(ci, t0, L):
            pk2 = pk2s[ci % 2]
            # 64-wide group reduce over i = (vr,vc); code is monotone over
            # the full 64 so no separate vr-combine is needed.
            nc.vector.tensor_reduce(A(r4, 0, 128, 0, [[1, 4 * L]]),
                                    A(pk2, 0, 128, 0, [[64, 4 * L], [1, 64]]),
                                    AX.X, AL.max)
            # decode: v = 127 - 64*packed; r4 flat (t, q, uh)
            tc0, rr0 = (t0 - 1) >> 5, (t0 - 1) & 31
            nc.scalar.activation(
                A(bpu, 0, 128, 64 * tc0 + rr0, [[1, L], [1024, 2], [32, 2]]),
                A(r4, 0, 128, 0, [[1, 4 * L]]),
                ACT.Copy, bias=127.0, scale=-64.0)

        pending = None  # (ci, t0, L) started but not finished
        for t in range(1, 513):
            slot = (t - 1) % 16
            sbase = slot * 256
            if t <= 511:
                in0 = A(crep, 0, 128, 0, [[0, 2], [1, 128]])
            else:
                in0 = A(zcol, 0, 128, 0, [[0, 2], [0, 128]])
            in1 = A(gam2, 0, 128, t - 1, [[2048, 2], [512, 4], [0, 32]])
            nc.vector.tensor_tensor(out=ssring[:, sbase:sbase + 256],
                                    in0=in0, in1=in1, op=AL.add)
            rr = spool.tile([128, 8], dt.float32, tag="rr")
            nc.vector.tensor_reduce(rr[:],
                                    A(ssring, 0, 128, sbase, [[32, 8], [1, 32]]),
                                    AX.X, AL.max, apply_transpose=True)
            # rr cols: (q, uh, vr)
            bsl = A(beta, 0, 128, t, [[2 * TB, 2], [TB, 2]])
            nc.vector.tensor_tensor(out=bsl,
                                    in0=A(rr, 0, 128, 0, [[4, 2], [2, 2]]),
                                    in1=A(rr, 0, 128, 1, [[4, 2], [2, 2]]),
                                    op=AL.max)
            if t <= 511:
                nc.vector.tensor_tensor(
                    out=A(gam2, 0, 128, t,
                          [[2048, 2], [512, 2], [1024, 2]]),
                    in0=A(beta, 0, 128, t, [[2 * TB, 2], [TB, 2], [0, 2]]),
                    in1=A(pot, 0, 128, t, [[1024, 2], [512, 2], [0, 2]]),
                    op=AL.add)

            if t % 8 == 0 and t <= 504:
                ci = t // 8 - 1
                if pending is not None:
                    _bp_finish(*pending)
                _bp_start(ci, t - 7, 8)
                pending = (ci, t - 7, 8)
            elif t == 511:
                if pending is not None:
                    _bp_finish(*pending)
                _bp_start(63, 505, 7)
                pending = (63, 505, 7)
            elif t == 512:
                _bp_finish(*pending)
                _bp_start(64, 512, 1)
                _bp_finish(64, 512, 1)
                pending = None

        # ---- Phase 4a: bulk transpose of bpu + SBUF->SBUF scatter ----
        # btp_all[(j,rr), 1024q + 64tc + 32uh + uc] = bp[b=(q,j),
        #   t=32tc+rr+1, u=32uh+uc]: per (j, rr, q) a contiguous 1024-run.
        nc.vector.transpose(out=btp_all[:], in_=bpu[:])
        brp = brow[:].ap[0][0]
        btpp = btp_all[:].ap[0][0]
        for b in range(NB):
            q, j = b >> 2, b & 3
            src = bass.AP(btp_all[:].tensor,
                          btp_all[:].offset + 32 * j * btpp + 1024 * q,
                          [[btpp, 32], [1, 1024]])
            dst = bass.AP(brow[:].tensor,
                          brow[:].offset + 16 * b * brp,
                          [[brp, 1], [1024, 32], [1, 1024]])
            nc.sync.dma_start(dst, src)

        def block_slice(t):
            off = (brow[:].offset + ((t - 1) & 31) * 1024
                   + ((t - 1) >> 5) * 64)
            return bass.AP(brow[:].tensor, off, [[brp, 128], [1, 64]])

        # ---- Phase 4b: backtrace chain (gpsimd) ----
        # chain: tags16 is 4-wide per t (indirect_copy needs out free >= 4;
        # only column 0 of each group is meaningful)
        nc.gpsimd.indirect_copy(tags16[:, 4 * 511:4 * 512], block_slice(512),
                                zidx[:], True)
        for t in range(511, 0, -1):
            nc.gpsimd.indirect_copy(tags16[:, 4 * (t - 1):4 * t],
                                    block_slice(t),
                                    tags16[:, 4 * t:4 * t + 1], True)

        if DEBUG:
            nc.sync.dma_start(d_gam[:], gam2[:])
            nc.sync.dma_start(d_beta[:], beta[:])
            nc.sync.dma_start(d_pot[:], pot[:])

        t16 = tags16[:]
        nc.vector.tensor_copy(
            out=tagsi[:],
            in_=bass.AP(t16.tensor, t16.offset, [[t16.ap[0][0], 128], [4, 512]]))
        ti = tagsi[:]
        src = bass.AP(ti.tensor, ti.offset, [[16 * ti.ap[0][0], 8], [1, 512]])
        nc.sync.dma_start(d_tags[:], src)

    nc.finalize()
    return nc


def _host_inputs(x, kernel, bias, chain_kernel, left_boundary, right_boundary):
    """Build per-core input maps (host does only sharding/layout)."""
    x = np.asarray(x, dtype=np.float32)
    wk = np.asarray(kernel, dtype=np.float32)
    bias = np.asarray(bias, dtype=np.float32)
    C = np.asarray(chain_kernel, dtype=np.float32)
    lb = np.asarray(left_boundary, dtype=np.float32)
    rb = np.asarray(right_boundary, dtype=np.float32)

    wkp = wk.reshape(8, 128, 64).transpose(1, 0, 2).reshape(128, 512).copy()
    # crep[32j+vc, 64uh+32vr+uc] = C[32vr+vc, 32uh+uc]
    crep = np.tile(
        C.reshape(2, 32, 2, 32).transpose(1, 2, 0, 3).reshape(32, 128),
        (4, 1)).copy()
    iota_row = 1.0 + (63.0 - np.arange(64, dtype=np.float32)) / 64.0
    iot = np.tile(iota_row[None, :], (128, 1)).copy()
    biasc = bias.reshape(64, 1).copy()
    lbc = lb.reshape(64, 1).copy()
    rbc = rb.reshape(64, 1).copy()

    in_maps = []
    for c in range(NCORE):
        xc = x[c * NB:(c + 1) * NB]                       # [8, 512, 1024]
        xt = np.ascontiguousarray(xc.transpose(0, 2, 1))  # [8, 1024, 512]
        in_maps.append({
            "xt": xt, "wkp": wkp, "biasc": biasc, "lbc": lbc, "rbc": rbc,
            "crep": crep, "iot": iot,
        })
    return in_maps


def kernel(x, kernel, bias, chain_kernel, left_boundary, right_boundary):
    from concourse.bass_utils import run_bass_kernel_spmd

    if "nc" not in _CACHE:
        _CACHE["nc"] = _build_program()
    nc = _CACHE["nc"]

    in_maps = _host_inputs(x, kernel, bias, chain_kernel,
                           left_boundary, right_boundary)
    res = run_bass_kernel_spmd(nc, in_maps, core_ids=list(range(NCORE)))
    outs = [np.asarray(r["tags"]).astype(np.int32) for r in res.results]
    return np.concatenate(outs, axis=0)


# revision 21
# speedup vs baseline: 1.0417x; 1.0377x over previous
"""CRF (dense projection + Viterbi decode) on 8 Trainium2 NeuronCores.

Strategy: data-parallel over batch (8 batches per core).
Per core:
  Phase 1: potentials = x @ W + bias (+boundary cols) on the PE, output in
           [u-partition, t-free] orientation (x fed pre-transposed from host).
  Phase 2: Viterbi forward scan, all-DVE, using tensor_reduce with
           apply_transpose (32x32 reshape-block) to reduce over the
           transition-source tag axis that lives on partitions.
           Layout: partition = (j=batch%4, vc=tag&31), free = (q=batch//4,
           vr=tag>>5, ...).
  Phase 3: backpointers recomputed in bulk (t-chunks of 8): recompute scores
           bitwise-identically, 32x32-block stream transpose, compare against
           stored per-step maxima, encode argmin-index via a monotone iota
           code, grouped max-reduce, decode to uint16. Chunk start (transpose,
           compare, Pool code-mult) and finish (reduce, decode) are emitted 8
           steps apart so the Pool mult never stalls the DVE pipeline.
  Phase 4: bulk 32x32 stream-transpose of the backpointer tile, then one
           SBUF->SBUF scatter DMA per (batch, tag-half) into per-batch
           partition rows; a sequential gpsimd indirect_copy chain walks the
           backpointers (one core-group of 16 partitions per batch).

All DMAs are issued from the SP/Activation sequencers (hardware DGE) so no
compute engine burns time on software descriptor generation.

Host side only shards/reshapes/transposes inputs and concatenates outputs.
"""

import numpy as np
from contextlib import ExitStack

B, T, F, U = 64, 512, 1024, 64
NB = 8  # batches per core
NCORE = 8

_CACHE = {}


def _build_program():
    import concourse.bass as bass
    import concourse.bacc as bacc
    import concourse.mybir as mybir
    import concourse.tile as tile

    dt = mybir.dt
    AL = mybir.AluOpType
    AX = mybir.AxisListType
    ACT = mybir.ActivationFunctionType

    nc = bacc.Bacc("TRN2", target_bir_lowering=False, debug=False,
                   num_devices=NCORE)

    # ---- DRAM I/O ----
    d_xt = nc.dram_tensor("xt", [NB, F, T], dt.float32, kind="ExternalInput")
    d_wkp = nc.dram_tensor("wkp", [128, 512], dt.float32, kind="ExternalInput")
    d_bias = nc.dram_tensor("biasc", [64, 1], dt.float32, kind="ExternalInput")
    d_lb = nc.dram_tensor("lbc", [64, 1], dt.float32, kind="ExternalInput")
    d_rb = nc.dram_tensor("rbc", [64, 1], dt.float32, kind="ExternalInput")
    d_crep = nc.dram_tensor("crep", [128, 128], dt.float32, kind="ExternalInput")
    d_iot = nc.dram_tensor("iot", [128, 64], dt.float32, kind="ExternalInput")
    d_tags = nc.dram_tensor("tags", [NB, T], dt.int32, kind="ExternalOutput")
    DEBUG = bool(int(__import__("os").environ.get("CRF_DEBUG", "0")))
    if DEBUG:
        d_gam = nc.dram_tensor("dbg_gam", [128, 4096], dt.float32,
                               kind="ExternalOutput")
        d_beta = nc.dram_tensor("dbg_beta", [128, 2052], dt.float32,
                                kind="ExternalOutput")
        d_pot = nc.dram_tensor("dbg_pot", [128, 2048], dt.float32,
                               kind="ExternalOutput")

    TB = 513          # beta t-slots (1..512 used)

    with tile.TileContext(nc) as tc, ExitStack() as ctx:
        cpool = ctx.enter_context(tc.tile_pool(name="consts", bufs=1))
        st = ctx.enter_context(tc.tile_pool(name="state", bufs=1))
        xpool = ctx.enter_context(tc.tile_pool(name="xs", bufs=8))
        ppool = ctx.enter_context(tc.tile_pool(name="ps", bufs=8, space="PSUM"))
        spool = ctx.enter_context(tc.tile_pool(name="scan", bufs=2))
        vpool = ctx.enter_context(tc.tile_pool(name="conv", bufs=2))

        wk = cpool.tile([128, 512], dt.float32, tag="wk")
        crep = cpool.tile([128, 128], dt.float32, tag="crep")
        iot = cpool.tile([128, 64], dt.float32, tag="iot")
        biasc = cpool.tile([64, 1], dt.float32, tag="biasc")
        lbc = cpool.tile([64, 1], dt.float32, tag="lbc")
        rbc = cpool.tile([64, 1], dt.float32, tag="rbc")
        zcol = cpool.tile([128, 1], dt.float32, tag="zcol")
        zidx = cpool.tile([128, 1], dt.uint16, tag="zidx")

        pot = st.tile([128, 2048], dt.float32, tag="pot")    # (q,uh,t<512)
        # gam2[(j,x), 2048q + 1024uh + 512vr + t] = gamma_t[b=(q,j),
        #   tag=(vr,x)] replicated over uh so the scan add reads (uh,vr) as
        #   one stride-512 dim.
        gam2 = st.tile([128, 4096], dt.float32, tag="gam2")
        beta = st.tile([128, 2 * 2 * TB], dt.float32, tag="beta")  # (q,uh,t<=512)
        # bpu[(j,uc), 1024q + 64*((t-1)>>5) + 32uh + ((t-1)&31)] =
        #   bp[b=(q,j), t, u=32uh+uc], t in 1..512 (all 2048 slots used)
        bpu = st.tile([128, 2048], dt.uint16, tag="bpu")
        # ssring slot cols: 128q + 64uh + 32vr + uc (per-step scores)
        ssring = st.tile([128, 16 * 256], dt.float32, tag="ssring")
        stts = [st.tile([128, 2048], dt.float32, tag=f"stt{i}", name=f"stt{i}")
                for i in range(2)]
        pkks = [st.tile([128, 2048], dt.bfloat16, tag=f"pkk{i}", name=f"pkk{i}")
                for i in range(2)]
        pk2s = [st.tile([128, 2048], dt.bfloat16, tag=f"pk2{i}", name=f"pk2{i}")
                for i in range(2)]
        r4 = st.tile([128, 64], dt.float32, tag="r4")
        btp_all = st.tile([128, 2048], dt.uint16, tag="btp_all")
        # brow[16b, 1024*((t-1)&31) + 64*((t-1)>>5) + u] = bp[b, t, u]
        brow = st.tile([128, 32 * 1024], dt.uint16, tag="brow")
        tags16 = st.tile([128, 4 * 512], dt.uint16, tag="tags16")
        tagsi = st.tile([128, 512], dt.int32, tag="tagsi")

        def A(tl, p0, np_, f0, dims):
            full = tl[:]
            pitch = full.ap[0][0]
            return bass.AP(full.tensor, full.offset + p0 * pitch + f0,
                           [[pitch, np_]] + [list(d) for d in dims])

        # ---- constant loads / inits (HWDGE via SP) ----
        nc.sync.dma_start(wk[:], d_wkp[:])
        nc.sync.dma_start(crep[:], d_crep[:])
        nc.sync.dma_start(iot[:], d_iot[:])
        nc.sync.dma_start(biasc[:], d_bias[:])
        nc.sync.dma_start(lbc[:], d_lb[:])
        nc.sync.dma_start(rbc[:], d_rb[:])
        nc.vector.memset(zcol[:], 0.0)
        nc.vector.memset(zidx[:], 0)
        nc.vector.memset(tags16[:], 0)
        nc.gpsimd.memset(brow[:], 0)
        if DEBUG:
            nc.vector.memset(beta[:], 0.0)

        # ---- Phase 1: potentials ----
        # PE warmup; also funnels the wk-DMA dependency into PE program order
        # so later matmuls need only their x-tile DMA wait (PE matmul HW
        # decode supports a single sync-wait).
        for b in range(NB):
            q, j = b >> 2, b & 3
            pp = ppool.tile([64, 512], dt.float32, tag="pp")
            if b == 0:
                nc.tensor.matmul(pp[0:64, 0:1], wk[:, 0:64], wk[:, 0:1],
                                 start=True, stop=True)
            for kc in range(8):
                xt = xpool.tile([128, 512], dt.float32, tag="xt")
                nc.sync.dma_start(xt[:], d_xt[b, kc * 128:(kc + 1) * 128, :])
                nc.tensor.matmul(pp[:], wk[:, kc * 64:(kc + 1) * 64], xt[:],
                                 start=(kc == 0), stop=(kc == 7))
            potb = vpool.tile([64, 512], dt.float32, tag="potb")
            nc.vector.tensor_scalar_add(potb[:], pp[:], biasc[:])
            nc.vector.tensor_add(potb[:, 0:1], potb[:, 0:1], lbc[:])
            nc.vector.tensor_add(potb[:, 511:512], potb[:, 511:512], rbc[:])
            for uh in range(2):
                dst = A(pot, 32 * j, 32, 1024 * q + 512 * uh, [[1, 512]])
                nc.scalar.dma_start(dst, potb[32 * uh:32 * uh + 32, :])

        # gamma_0 = pot_0 (replicated over uh)
        nc.vector.tensor_copy(
            out=A(gam2, 0, 128, 0, [[2048, 2], [512, 2], [1024, 2]]),
            in_=A(pot, 0, 128, 0, [[1024, 2], [512, 2], [0, 2]]))

        # ---- Phase 2: forward scan with split-pipelined bp chunks ----
        # Each chunk is processed in two independent qq-halves so the DVE
        # half-reduce only waits on one Pool code-mult at a time.
        def _bp_start(ci, t0, L):
            pkk = pkks[ci % 2]
            stt = stts[ci % 2]
            rbase = ((t0 - 1) % 16) * 256
            for qq in range(2):
                off = qq * 128
                # 32-block transpose of this qq-half of the ring chunk:
                # stt layout becomes (t, q, uh, i), i = 32vr + former
                # partition index
                nc.vector.transpose(
                    out=A(stt, 0, 128, off, [[256, L], [1, 128]]),
                    in_=A(ssring, 0, 128, rbase + off, [[256, L], [1, 128]]))
                s_in = A(stt, 0, 128, off, [[256, L], [64, 2], [1, 64]])
                b_in = A(beta, 0, 128, 2 * TB * qq + t0,
                         [[1, L], [TB, 2], [0, 64]])
                p_out = A(pkk, 0, 128, off, [[256, L], [64, 2], [1, 64]])
                nc.vector.tensor_tensor(out=p_out, in0=s_in, in1=b_in,
                                        op=AL.is_ge)
                i_in = A(iot, 0, 128, 0, [[0, L], [0, 2], [1, 64]])
                s_out = A(pk2s[ci % 2], 0, 128, off,
                          [[256, L], [64, 2], [1, 64]])
                nc.gpsimd.tensor_tensor(out=s_out, in0=p_out, in1=i_in,
                                        op=AL.mult)

        def _bp_finish(ci, t0, L):
            pk2 = pk2s[ci % 2]
            # 64-wide group reduce over i = (vr,vc); code is monotone over
            # the full 64 so no separate vr-combine is needed. r4 flat
            # (t, q, uh); one half-reduce per qq.
            for qq in range(2):
                nc.vector.tensor_reduce(
                    A(r4, 0, 128, 2 * qq, [[4, L], [1, 2]]),
                    A(pk2, 0, 128, qq * 128, [[256, L], [64, 2], [1, 64]]),
                    AX.X, AL.max)
            # decode: v = 127 - 64*packed
            tc0, rr0 = (t0 - 1) >> 5, (t0 - 1) & 31
            nc.scalar.activation(
                A(bpu, 0, 128, 64 * tc0 + rr0, [[1, L], [1024, 2], [32, 2]]),
                A(r4, 0, 128, 0, [[1, 4 * L]]),
                ACT.Copy, bias=127.0, scale=-64.0)

        pending = None  # (ci, t0, L) started but not finished
        for t in range(1, 513):
            slot = (t - 1) % 16
            sbase = slot * 256
            if t <= 511:
                in0 = A(crep, 0, 128, 0, [[0, 2], [1, 128]])
            else:
                in0 = A(zcol, 0, 128, 0, [[0, 2], [0, 128]])
            in1 = A(gam2, 0, 128, t - 1, [[2048, 2], [512, 4], [0, 32]])
            nc.vector.tensor_tensor(out=ssring[:, sbase:sbase + 256],
                                    in0=in0, in1=in1, op=AL.add)
            rr = spool.tile([128, 8], dt.float32, tag="rr")
            nc.vector.tensor_reduce(rr[:],
                                    A(ssring, 0, 128, sbase, [[32, 8], [1, 32]]),
                                    AX.X, AL.max, apply_transpose=True)
            # rr cols: (q, uh, vr)
            bsl = A(beta, 0, 128, t, [[2 * TB, 2], [TB, 2]])
            nc.vector.tensor_tensor(out=bsl,
                                    in0=A(rr, 0, 128, 0, [[4, 2], [2, 2]]),
                                    in1=A(rr, 0, 128, 1, [[4, 2], [2, 2]]),
                                    op=AL.max)
            if t <= 511:
                nc.vector.tensor_tensor(
                    out=A(gam2, 0, 128, t,
                          [[2048, 2], [512, 2], [1024, 2]]),
                    in0=A(beta, 0, 128, t, [[2 * TB, 2], [TB, 2], [0, 2]]),
                    in1=A(pot, 0, 128, t, [[1024, 2], [512, 2], [0, 2]]),
                    op=AL.add)

            if t % 8 == 0 and t <= 504:
                ci = t // 8 - 1
                if pending is not None:
                    _bp_finish(*pending)
                _bp_start(ci, t - 7, 8)
                pending = (ci, t - 7, 8)
            elif t == 511:
                if pending is not None:
                    _bp_finish(*pending)
                _bp_start(63, 505, 7)
                pending = (63, 505, 7)
            elif t == 512:
                _bp_finish(*pending)
                _bp_start(64, 512, 1)
                _bp_finish(64, 512, 1)
                pending = None

        # ---- Phase 4a: bulk transpose of bpu + SBUF->SBUF scatter ----
        # btp_all[(j,rr), 1024q + 64tc + 32uh + uc] = bp[b=(q,j),
        #   t=32tc+rr+1, u=32uh+uc]: per (j, rr, q) a contiguous 1024-run.
        nc.vector.transpose(out=btp_all[:], in_=bpu[:])
        brp = brow[:].ap[0][0]
        btpp = btp_all[:].ap[0][0]
        for b in range(NB):
            q, j = b >> 2, b & 3
            src = bass.AP(btp_all[:].tensor,
                          btp_all[:].offset + 32 * j * btpp + 1024 * q,
                          [[btpp, 32], [1, 1024]])
            dst = bass.AP(brow[:].tensor,
                          brow[:].offset + 16 * b * brp,
                          [[brp, 1], [1024, 32], [1, 1024]])
            nc.sync.dma_start(dst, src)

        def block_slice(t):
            off = (brow[:].offset + ((t - 1) & 31) * 1024
                   + ((t - 1) >> 5) * 64)
            return bass.AP(brow[:].tensor, off, [[brp, 128], [1, 64]])

        # ---- Phase 4b: backtrace chain (gpsimd) ----
        # chain: tags16 is 4-wide per t (indirect_copy needs out free >= 4;
        # only column 0 of each group is meaningful)
        nc.gpsimd.indirect_copy(tags16[:, 4 * 511:4 * 512], block_slice(512),
                                zidx[:], True)
        for t in range(511, 0, -1):
            nc.gpsimd.indirect_copy(tags16[:, 4 * (t - 1):4 * t],
                                    block_slice(t),
                                    tags16[:, 4 * t:4 * t + 1], True)

        if DEBUG:
            nc.sync.dma_start(d_gam[:], gam2[:])
            nc.sync.dma_start(d_beta[:], beta[:])
            nc.sync.dma_start(d_pot[:], pot[:])

        t16 = tags16[:]
        nc.vector.tensor_copy(
            out=tagsi[:],
            in_=bass.AP(t16.tensor, t16.offset, [[t16.ap[0][0], 128], [4, 512]]))
        ti = tagsi[:]
        src = bass.AP(ti.tensor, ti.offset, [[16 * ti.ap[0][0], 8], [1, 512]])
        nc.sync.dma_start(d_tags[:], src)

    nc.finalize()
    return nc


def _host_inputs(x, kernel, bias, chain_kernel, left_boundary, right_boundary):
    """Build per-core input maps (host does only sharding/layout)."""
    x = np.asarray(x, dtype=np.float32)
    wk = np.asarray(kernel, dtype=np.float32)
    bias = np.asarray(bias, dtype=np.float32)
    C = np.asarray(chain_kernel, dtype=np.float32)
    lb = np.asarray(left_boundary, dtype=np.float32)
    rb = np.asarray(right_boundary, dtype=np.float32)

    wkp = wk.reshape(8, 128, 64).transpose(1, 0, 2).reshape(128, 512).copy()
    # crep[32j+vc, 64uh+32vr+uc] = C[32vr+vc, 32uh+uc]
    crep = np.tile(
        C.reshape(2, 32, 2, 32).transpose(1, 2, 0, 3).reshape(32, 128),
        (4, 1)).copy()
    iota_row = 1.0 + (63.0 - np.arange(64, dtype=np.float32)) / 64.0
    iot = np.tile(iota_row[None, :], (128, 1)).copy()
    biasc = bias.reshape(64, 1).copy()
    lbc = lb.reshape(64, 1).copy()
    rbc = rb.reshape(64, 1).copy()

    in_maps = []
    for c in range(NCORE):
        xc = x[c * NB:(c + 1) * NB]                       # [8, 512, 1024]
        xt = np.ascontiguousarray(xc.transpose(0, 2, 1))  # [8, 1024, 512]
        in_maps.append({
            "xt": xt, "wkp": wkp, "biasc": biasc, "lbc": lbc, "rbc": rbc,
            "crep": crep, "iot": iot,
        })
    return in_maps


def kernel(x, kernel, bias, chain_kernel, left_boundary, right_boundary):
    from concourse.bass_utils import run_bass_kernel_spmd

    if "nc" not in _CACHE:
        _CACHE["nc"] = _build_program()
    nc = _CACHE["nc"]

    in_maps = _host_inputs(x, kernel, bias, chain_kernel,
                           left_boundary, right_boundary)
    res = run_bass_kernel_spmd(nc, in_maps, core_ids=list(range(NCORE)))
    outs = [np.asarray(r["tags"]).astype(np.int32) for r in res.results]
    return np.concatenate(outs, axis=0)
